# revision 21
# baseline (speedup 1.0000x reference)
"""Trainium2 Bass kernel for the bidirectional diagonal-SSM kernel generator.

Computes, for inputs log_dt [H], log_a_real [H,N], a_imag [H,N],
coeffs [2,H,N,2] (H=1024, N=32, L=4096):

    dt    = exp(log_dt)
    a     = -exp(log_a_real) + i*a_imag
    da    = a * dt[:,None]
    sc    = (coeffs[...,0] + i*coeffs[...,1]) * (exp(da)-1)/a     # [2,H,N]
    out[d,h,l] = 2*Re( sum_n sc[d,h,n] * exp(da[h,n]*l) )        # [2,H,L] f32

Sharding: d_model (H) split across 8 cores, 128 channels each; no
cross-core communication.

Strategy (v3: data-adaptive output truncation, 64-wide basis, flipped
matmuls in the baseline's proven quadrant pattern):

  * l = 64*q + j decomposition: out[d,h,64q+j] = sum_{n,cs}
    B[(n,cs), j] * W[(n,cs), (q,d)], with B = exp(da*j) (j<64) packed
    as Re/Im rows and W = Re/-Im of sc*exp(da*64q), host-precomputed
    in f16.  The 64-wide basis halves the untruncatable B traffic
    relative to the 128-wide split.
  * The SSM kernels decay geometrically (|exp(da)| < 1).  The host
    bounds each channel's truncation tail EXACTLY via geometric pole
    sums and keeps only Q[h] of the 64 output 64-blocks such that
    total truncation error < TRUNC_GAMMA * ||out||_F; dropped blocks
    are zero-filled on the host.  Keeps ~45-50% of W/output traffic,
    PSUM, evac and store work on the reference data.
  * FLIPPED matmul: per channel, stationary lhsT = B_ch [K=64 (n,cs),
    M=64 j], moving rhs = W_ch [64, 2*Q] -> PSUM out [64 j-partitions,
    2*Q cols].  Truncation lands on the matmul FREE dim.  Channel
    pairs use the baseline's quadrant pattern: ch A = PE rows 0:63 x
    cols 0:63 -> PSUM partitions 0:64, ch B = rows 64:127 x cols
    64:127 -> partitions 64:128.  The two matmuls run concurrently in
    disjoint quadrants and write disjoint PSUM partitions (same cols)
    - no PSUM write-port conflict.
  * Channels sorted by descending Q (host unscrambles); all 8 cores
    share ONE traced module built for the elementwise-max profile.
  * Pair outputs pack greedily into PSUM banks (<=512 f32 cols); one
    evac copy per bank (f32->f16) alternating ScalarE/VectorE; stores
    ride the gpsimd SWDGE queue, last two groups on sync.
  * Per-chunk loads combine B and truncated W contiguously, split
    across the two HWDGE rings, all dispatched up-front.

Per-core traffic: ~1.05 MB basis + ~1 MB weights in, ~1 MB out
(data-dependent), vs 3 MB in + 2 MB out for the dense baseline.
"""

import sys

import numpy as np

sys.path.insert(0, "/opt/trn_rl_repo")

from contextlib import ExitStack

from concourse import bacc, mybir, tile
from concourse.bass_utils import run_bass_kernel_spmd

H = 1024          # d_model
NPOLE = 32        # poles per channel
L = 4096          # sequence length
NDIR = 2          # directions
NCORES = 8
HC = H // NCORES  # channels per core = 128
PAIRS = HC // 2   # channel pairs per core = 64

BW = 64           # j range (basis width)
NQ = L // BW      # q range = 64
TRUNC_GAMMA = 4e-3   # truncation budget as fraction of ||out||_F
PSUM_COLS = 512      # f32 cols per PSUM bank
# load chunk boundaries as cumulative BYTE fractions (pairs rounded):
# small first chunk -> early first matmul; small last chunk -> early
# final matmuls/evacs
CHUNK_FRACS = [0.025, 0.25, 0.47, 0.67, 0.87, 1.0]
# evac groups per store DMA: pairs of groups pipeline well; a lone
# final group keeps the tail store tiny
STORE_GROUPS = [2, 2, 2, 2, 1]

F32 = mybir.dt.float32
F16 = mybir.dt.float16


def _chan_plan(log_dt, log_a_real, a_imag, coeffs):
    """Per-channel kept 64-blocks Q [H] via an exact tail-norm bound."""
    dt = np.exp(log_dt.astype(np.float64))
    a = -np.exp(log_a_real.astype(np.float64)) + 1j * a_imag.astype(np.float64)
    da = a * dt[:, None]                                     # [H,N]
    z = np.exp(da)
    c = coeffs[..., 0].astype(np.float64) + 1j * coeffs[..., 1].astype(np.float64)
    sc2 = 2.0 * c * (z - 1.0) / a                            # [2,H,N]

    # K(l) = Re(S), S = sum_n sc2 z^l; sum_l K^2 = sum_l (S^2+2|S|^2+S̄^2)/4
    # tail2(L0) = sum_{l>=L0}^{L} K^2 computed with geometric pole sums;
    # iterate pw = w^(64k) to get all 64 candidate cuts cheaply.
    zz = (z[:, :, None] * z[:, None, :]).reshape(H, -1)      # [H,N*N]
    zzc = (z[:, :, None] * np.conj(z)[:, None, :]).reshape(H, -1)
    tail2 = np.zeros((H, NQ))
    head = 0.0
    for d in range(NDIR):
        s = sc2[d]
        pp = (s[:, :, None] * s[:, None, :]).reshape(H, -1)
        pc = (s[:, :, None] * np.conj(s)[:, None, :]).reshape(H, -1)
        for w, coef in ((zz, pp), (zzc, pc)):
            A = coef / (1.0 - w)                             # [H,N*N]
            wL = w ** L
            wstep = w ** BW
            const = (A * wL).sum(axis=1)                     # subtractive part
            head += 0.5 * ((A.sum(axis=1) - const).real.sum())
            pw = wstep.copy()
            for k in range(NQ):
                tail2[:, k] += 0.5 * ((A * pw).sum(axis=1) - const).real
                if k + 1 < NQ:
                    pw *= wstep
    np.maximum(tail2, 0.0, out=tail2)
    norm2 = float(max(head, 1e-30))

    budget2 = (TRUNC_GAMMA ** 2) * norm2 / H                 # per channel
    Q = np.full(H, NQ, np.int64)
    ok = tail2 <= budget2
    for h in range(H):
        idx = np.nonzero(ok[h])[0]
        if idx.size:
            Q[h] = idx[0] + 1
    return Q, da, sc2


def _host_prep(log_dt, log_a_real, a_imag, coeffs):
    """Returns (per-core comb arrays, shared layout, per-core chan order)."""
    Q, da, sc2 = _chan_plan(log_dt, log_a_real, a_imag, coeffs)

    j = np.arange(BW, dtype=np.float64)
    zB = np.exp(da[:, :, None] * j)                          # [H,N,BW]
    basis = np.stack([zB.real, zB.imag], axis=2)             # [H,N,2,BW]
    basis = basis.reshape(H, 2 * NPOLE, BW).astype(np.float16)

    q = BW * np.arange(NQ, dtype=np.float64)
    zA = np.exp(da[:, :, None] * q)                          # [H,N,NQ]
    G = sc2[:, :, :, None] * zA[None]                        # [2,H,N,NQ]
    # W[h, (n,cs), (q,d)]: cs=0 -> Re, cs=1 -> -Im; col = q*2 + d
    w_all = np.stack([G.real, -G.imag], axis=3)              # [2,H,N,2,NQ]
    w_all = (w_all.transpose(1, 2, 3, 4, 0)
             .reshape(H, 2 * NPOLE, NQ * 2).astype(np.float16))

    chans_per_core = []
    qpair_per_core = np.zeros((NCORES, PAIRS), np.int64)
    for core in range(NCORES):
        hs = slice(core * HC, (core + 1) * HC)
        order = np.argsort(-Q[hs], kind="stable")
        chans = core * HC + order
        chans_per_core.append(chans)
        for p in range(PAIRS):
            qpair_per_core[core, p] = max(Q[chans[2 * p]],
                                          Q[chans[2 * p + 1]])
    qpair = qpair_per_core.max(axis=0)                       # shared profile
    wcols = 2 * qpair                                        # W cols/channel

    offs = np.concatenate([[0], np.cumsum(BW + wcols)])
    total_cols = int(offs[-1])
    combs = []
    for core in range(NCORES):
        chans = chans_per_core[core]
        comb = np.zeros((128, total_cols), np.float16)
        for p in range(PAIRS):
            o, wc = int(offs[p]), int(wcols[p])
            ha, hb = chans[2 * p], chans[2 * p + 1]
            comb[0:64, o:o + BW] = basis[ha]
            comb[64:128, o:o + BW] = basis[hb]
            comb[0:64, o + BW:o + BW + wc] = w_all[ha][:, :wc]
            comb[64:128, o + BW:o + BW + wc] = w_all[hb][:, :wc]
        combs.append(np.ascontiguousarray(comb))
    layout = dict(qpair=qpair, wcols=wcols, offs=offs, total_cols=total_cols)
    return combs, layout, chans_per_core


def _device_plan(layout):
    """Chunks (loads) and PSUM groups (pair col packing, wc cols/pair)."""
    wcols, offs = layout["wcols"], layout["offs"]
    groups = []
    p0, cols = 0, 0
    for p in range(PAIRS):
        need = int(wcols[p])
        if cols + need > PSUM_COLS:
            groups.append((p0, p - p0, cols))
            p0, cols = p, 0
        cols += need
    groups.append((p0, PAIRS - p0, cols))
    # chunk boundaries at byte fractions (pairs kept whole, monotone)
    total = float(offs[-1])
    bounds = [0]
    for f in CHUNK_FRACS:
        b = int(np.searchsorted(offs, f * total))
        b = max(b, bounds[-1] + 1)
        bounds.append(min(b, PAIRS))
    bounds[-1] = PAIRS
    chunks = []
    for p0, p1 in zip(bounds[:-1], bounds[1:]):
        if p1 > p0:
            chunks.append((p0, p1 - p0, int(offs[p0]), int(offs[p1])))
    return chunks, groups


def _build_module(layout):
    """Trace the Bass/Tile program (shared by all cores)."""
    wcols, offs = layout["wcols"], layout["offs"]
    chunks, groups = _device_plan(layout)
    total_cols = layout["total_cols"]
    out_cols = int(wcols.sum())

    nc = bacc.Bacc(None)
    comb_d = nc.declare_dram_parameter("comb", [128, total_cols], F16,
                                       isOutput=False)
    out_d = nc.declare_dram_parameter("out", [128, out_cols], F16,
                                      isOutput=True)

    with ExitStack() as ctx:
        tc = ctx.enter_context(tile.TileContext(nc))
        c_pool = ctx.enter_context(tc.tile_pool(name="c", bufs=len(chunks)))
        o_pool = ctx.enter_context(tc.tile_pool(name="o", bufs=4))
        psum_pool = ctx.enter_context(tc.tile_pool(name="psum", bufs=6,
                                                   space="PSUM"))

        cts = []
        for ci, (p0, np_, c0, c1) in enumerate(chunks):
            ct = c_pool.tile([128, c1 - c0], F16, tag=f"ct{ci}",
                             name=f"ct{ci}")
            eng = nc.sync if ci % 2 == 0 else nc.scalar
            eng.dma_start(ct[:], comb_d[:, c0:c1])
            cts.append(ct)

        chunk_of = {}
        for ci, (p0, np_, c0, c1) in enumerate(chunks):
            for p in range(p0, p0 + np_):
                chunk_of[p] = (ci, c0)

        # store schedule: STORE_GROUPS[k] evac groups per store DMA (padded
        # with single-group stores if needed); big early stores amortize
        # descriptor overhead, tiny late ones shorten the post-evac tail.
        # Engines rotate gpsimd/sync/scalar; the final store always rides
        # the by-then-idle sync ring.
        sg = list(STORE_GROUPS)
        while sum(sg) < len(groups):
            sg.append(1)
        while sum(sg) > len(groups) and sg:
            if sg[-1] > 1:
                sg[-1] -= 1
            else:
                sg.pop()
        store_of = []
        for si, n in enumerate(sg):
            store_of += [si] * n
        store_engs = [nc.gpsimd, nc.sync, nc.scalar]

        ocol = 0
        ot = None
        ot_cols = 0
        ot_base = 0
        for gi, (g0, gnp, gcols) in enumerate(groups):
            acc = psum_pool.tile([128, PSUM_COLS], F32, tag="acc", name="acc")
            ccol = 0
            for p in range(g0, g0 + gnp):
                ci, c0 = chunk_of[p]
                ct = cts[ci]
                o = int(offs[p]) - c0
                wc = int(wcols[p])
                # ch A: PE quadrant (0,0), PSUM partitions 0:64
                nc.tensor.matmul(acc[0:64, ccol:ccol + wc],
                                 ct[0:64, o:o + BW],
                                 ct[0:64, o + BW:o + BW + wc],
                                 start=True, stop=True)
                # ch B: PE quadrant (64,64), PSUM partitions 64:128
                nc.tensor.matmul(acc[64:128, ccol:ccol + wc],
                                 ct[64:128, o:o + BW],
                                 ct[64:128, o + BW:o + BW + wc],
                                 start=True, stop=True)
                ccol += wc
            if ot is None:
                nsg = sg[store_of[gi]]
                ot = o_pool.tile([128, nsg * PSUM_COLS], F16,
                                 tag=f"ot{nsg}", name="ot")
                ot_cols = 0
                ot_base = ocol
            # evac: vector for the final group (lower PSUM access latency
            # on the critical tail), else alternate scalar/vector
            if gi == len(groups) - 1 or gi % 2 == 1:
                nc.vector.tensor_copy(ot[:, ot_cols:ot_cols + gcols],
                                      acc[:, :gcols])
            else:
                nc.scalar.copy(ot[:, ot_cols:ot_cols + gcols],
                               acc[:, :gcols])
            ot_cols += gcols
            ocol += gcols
            if gi == len(groups) - 1 or store_of[gi + 1] != store_of[gi]:
                si = store_of[gi]
                eng = (nc.sync if gi == len(groups) - 1
                       else store_engs[si % len(store_engs)])
                eng.dma_start(out_d[:, ot_base:ot_base + ot_cols],
                              ot[:, :ot_cols])
                ot = None

    nc.finalize()
    return nc


def run(inputs, trace=False, **run_kwargs):
    """Run on 8 NeuronCores. Returns (full_output, BassKernelResults)."""
    log_dt = np.asarray(inputs["log_dt"], np.float32)
    log_a_real = np.asarray(inputs["log_a_real"], np.float32)
    a_imag = np.asarray(inputs["a_imag"], np.float32)
    coeffs = np.asarray(inputs["coeffs"], np.float32)
    seq_len = int(inputs.get("sequence_length", L))
    assert log_dt.shape == (H,) and log_a_real.shape == (H, NPOLE)
    assert a_imag.shape == (H, NPOLE) and coeffs.shape == (NDIR, H, NPOLE, 2)
    assert seq_len == L, f"kernel is compiled for sequence_length={L}"

    combs, layout, chans_per_core = _host_prep(
        log_dt, log_a_real, a_imag, coeffs)
    nc = _build_module(layout)
    in_maps = [{"comb": combs[c]} for c in range(NCORES)]
    results = run_bass_kernel_spmd(nc, in_maps, list(range(NCORES)),
                                   trace=trace, **run_kwargs)

    wcols = layout["wcols"]
    out = np.zeros((NDIR, H, L), np.float32)
    for core in range(NCORES):
        o = np.asarray(results.results[core]["out"], np.float32)
        chans = chans_per_core[core]
        ocol = 0
        for p in range(PAIRS):
            wc = int(wcols[p])
            q0 = wc // 2
            for k in range(2):
                h = chans[2 * p + k]
                blk = o[64 * k:64 * k + 64, ocol:ocol + wc]
                blk = blk.reshape(BW, q0, 2)
                out[:, h, :q0 * BW] = blk.transpose(2, 1, 0).reshape(2, -1)
            ocol += wc
    return out, results


def kernel(**inputs):
    return run(inputs)[0]


# revision 22
# speedup vs baseline: 1.0665x; 1.0665x over previous
"""Trainium2 Bass kernel for the bidirectional diagonal-SSM kernel generator.

Computes, for inputs log_dt [H], log_a_real [H,N], a_imag [H,N],
coeffs [2,H,N,2] (H=1024, N=32, L=4096):

    dt    = exp(log_dt)
    a     = -exp(log_a_real) + i*a_imag
    da    = a * dt[:,None]
    sc    = (coeffs[...,0] + i*coeffs[...,1]) * (exp(da)-1)/a     # [2,H,N]
    out[d,h,l] = 2*Re( sum_n sc[d,h,n] * exp(da[h,n]*l) )        # [2,H,L] f32

Sharding: d_model (H) split across 8 cores, 128 channels each; no
cross-core communication.

Strategy (v3: data-adaptive output truncation, 64-wide basis, flipped
matmuls in the baseline's proven quadrant pattern):

  * l = 64*q + j decomposition: out[d,h,64q+j] = sum_{n,cs}
    B[(n,cs), j] * W[(n,cs), (q,d)], with B = exp(da*j) (j<64) packed
    as Re/Im rows and W = Re/-Im of sc*exp(da*64q), host-precomputed
    in f16.  The 64-wide basis halves the untruncatable B traffic
    relative to the 128-wide split.
  * The SSM kernels decay geometrically (|exp(da)| < 1).  The host
    bounds each channel's truncation tail EXACTLY via geometric pole
    sums and keeps only Q[h] of the 64 output 64-blocks such that
    total truncation error < TRUNC_GAMMA * ||out||_F; dropped blocks
    are zero-filled on the host.  Keeps ~45-50% of W/output traffic,
    PSUM, evac and store work on the reference data.
  * FLIPPED matmul: per channel, stationary lhsT = B_ch [K=64 (n,cs),
    M=64 j], moving rhs = W_ch [64, 2*Q] -> PSUM out [64 j-partitions,
    2*Q cols].  Truncation lands on the matmul FREE dim.  Channel
    pairs use the baseline's quadrant pattern: ch A = PE rows 0:63 x
    cols 0:63 -> PSUM partitions 0:64, ch B = rows 64:127 x cols
    64:127 -> partitions 64:128.  The two matmuls run concurrently in
    disjoint quadrants and write disjoint PSUM partitions (same cols)
    - no PSUM write-port conflict.
  * Channels sorted by descending Q (host unscrambles); all 8 cores
    share ONE traced module built for the elementwise-max profile.
  * Pair outputs pack greedily into PSUM banks (<=512 f32 cols); one
    evac copy per bank (f32->f16) alternating ScalarE/VectorE; stores
    ride the gpsimd SWDGE queue, last two groups on sync.
  * Per-chunk loads combine B and truncated W contiguously, split
    across the two HWDGE rings, all dispatched up-front.

Per-core traffic: ~1.05 MB basis + ~1 MB weights in, ~1 MB out
(data-dependent), vs 3 MB in + 2 MB out for the dense baseline.
"""

import sys

import numpy as np

sys.path.insert(0, "/opt/trn_rl_repo")

from contextlib import ExitStack

from concourse import bacc, mybir, tile
from concourse.bass_utils import run_bass_kernel_spmd

H = 1024          # d_model
NPOLE = 32        # poles per channel
L = 4096          # sequence length
NDIR = 2          # directions
NCORES = 8
HC = H // NCORES  # channels per core = 128
PAIRS = HC // 2   # channel pairs per core = 64

BW = 64           # j range (basis width)
NQ = L // BW      # q range = 64
TRUNC_GAMMA = 2e-3   # truncation budget as fraction of ||out||_F
PSUM_COLS = 512      # f32 cols per PSUM bank
# load chunk boundaries as cumulative BYTE fractions (pairs rounded):
# small first chunk -> early first matmul; small last chunk -> early
# final matmuls/evacs
CHUNK_FRACS = [0.045, 0.25, 0.47, 0.67, 0.87, 1.0]
# evac groups per store DMA: pairs of groups pipeline well; a lone
# final group keeps the tail store tiny
STORE_GROUPS = [2, 2, 2, 2, 1]

F32 = mybir.dt.float32
F16 = mybir.dt.float16


def _chan_plan(log_dt, log_a_real, a_imag, coeffs):
    """Per-channel kept 64-blocks Q [H] via an exact tail-norm bound."""
    dt = np.exp(log_dt.astype(np.float64))
    a = -np.exp(log_a_real.astype(np.float64)) + 1j * a_imag.astype(np.float64)
    da = a * dt[:, None]                                     # [H,N]
    z = np.exp(da)
    c = coeffs[..., 0].astype(np.float64) + 1j * coeffs[..., 1].astype(np.float64)
    sc2 = 2.0 * c * (z - 1.0) / a                            # [2,H,N]

    # K(l) = Re(S), S = sum_n sc2 z^l; sum_l K^2 = sum_l (S^2+2|S|^2+S̄^2)/4
    # tail2(L0) = sum_{l>=L0}^{L} K^2 computed with geometric pole sums;
    # iterate pw = w^(64k) to get all 64 candidate cuts cheaply.
    zz = (z[:, :, None] * z[:, None, :]).reshape(H, -1)      # [H,N*N]
    zzc = (z[:, :, None] * np.conj(z)[:, None, :]).reshape(H, -1)
    tail2 = np.zeros((H, NQ))
    head = 0.0
    for d in range(NDIR):
        s = sc2[d]
        pp = (s[:, :, None] * s[:, None, :]).reshape(H, -1)
        pc = (s[:, :, None] * np.conj(s)[:, None, :]).reshape(H, -1)
        for w, coef in ((zz, pp), (zzc, pc)):
            A = coef / (1.0 - w)                             # [H,N*N]
            wL = w ** L
            wstep = w ** BW
            const = (A * wL).sum(axis=1)                     # subtractive part
            head += 0.5 * ((A.sum(axis=1) - const).real.sum())
            pw = wstep.copy()
            for k in range(NQ):
                tail2[:, k] += 0.5 * ((A * pw).sum(axis=1) - const).real
                if k + 1 < NQ:
                    pw *= wstep
    np.maximum(tail2, 0.0, out=tail2)
    norm2 = float(max(head, 1e-30))

    budget2 = (TRUNC_GAMMA ** 2) * norm2 / H                 # per channel
    Q = np.full(H, NQ, np.int64)
    ok = tail2 <= budget2
    for h in range(H):
        idx = np.nonzero(ok[h])[0]
        if idx.size:
            Q[h] = idx[0] + 1
    return Q, da, sc2


def _host_prep(log_dt, log_a_real, a_imag, coeffs):
    """Returns (per-core comb arrays, shared layout, per-core chan order)."""
    Q, da, sc2 = _chan_plan(log_dt, log_a_real, a_imag, coeffs)

    j = np.arange(BW, dtype=np.float64)
    zB = np.exp(da[:, :, None] * j)                          # [H,N,BW]
    basis = np.stack([zB.real, zB.imag], axis=2)             # [H,N,2,BW]
    basis = basis.reshape(H, 2 * NPOLE, BW).astype(np.float16)

    q = BW * np.arange(NQ, dtype=np.float64)
    zA = np.exp(da[:, :, None] * q)                          # [H,N,NQ]
    G = sc2[:, :, :, None] * zA[None]                        # [2,H,N,NQ]
    # W[h, (n,cs), (q,d)]: cs=0 -> Re, cs=1 -> -Im; col = q*2 + d
    w_all = np.stack([G.real, -G.imag], axis=3)              # [2,H,N,2,NQ]
    w_all = (w_all.transpose(1, 2, 3, 4, 0)
             .reshape(H, 2 * NPOLE, NQ * 2).astype(np.float16))

    chans_per_core = []
    qpair_per_core = np.zeros((NCORES, PAIRS), np.int64)
    for core in range(NCORES):
        hs = slice(core * HC, (core + 1) * HC)
        order = np.argsort(-Q[hs], kind="stable")
        chans = core * HC + order
        chans_per_core.append(chans)
        for p in range(PAIRS):
            qpair_per_core[core, p] = max(Q[chans[2 * p]],
                                          Q[chans[2 * p + 1]])
    qpair = qpair_per_core.max(axis=0)                       # shared profile
    wcols = 2 * qpair                                        # W cols/channel

    offs = np.concatenate([[0], np.cumsum(BW + wcols)])
    total_cols = int(offs[-1])
    combs = []
    for core in range(NCORES):
        chans = chans_per_core[core]
        comb = np.zeros((128, total_cols), np.float16)
        for p in range(PAIRS):
            o, wc = int(offs[p]), int(wcols[p])
            ha, hb = chans[2 * p], chans[2 * p + 1]
            comb[0:64, o:o + BW] = basis[ha]
            comb[64:128, o:o + BW] = basis[hb]
            comb[0:64, o + BW:o + BW + wc] = w_all[ha][:, :wc]
            comb[64:128, o + BW:o + BW + wc] = w_all[hb][:, :wc]
        combs.append(np.ascontiguousarray(comb))
    layout = dict(qpair=qpair, wcols=wcols, offs=offs, total_cols=total_cols)
    return combs, layout, chans_per_core


def _device_plan(layout):
    """Chunks (loads) and PSUM groups (pair col packing, wc cols/pair)."""
    wcols, offs = layout["wcols"], layout["offs"]
    groups = []
    p0, cols = 0, 0
    for p in range(PAIRS):
        need = int(wcols[p])
        if cols + need > PSUM_COLS:
            groups.append((p0, p - p0, cols))
            p0, cols = p, 0
        cols += need
    groups.append((p0, PAIRS - p0, cols))
    # chunk boundaries at byte fractions (pairs kept whole, monotone)
    total = float(offs[-1])
    bounds = [0]
    for f in CHUNK_FRACS:
        b = int(np.searchsorted(offs, f * total))
        b = max(b, bounds[-1] + 1)
        bounds.append(min(b, PAIRS))
    bounds[-1] = PAIRS
    chunks = []
    for p0, p1 in zip(bounds[:-1], bounds[1:]):
        if p1 > p0:
            chunks.append((p0, p1 - p0, int(offs[p0]), int(offs[p1])))
    return chunks, groups


def _build_module(layout):
    """Trace the Bass/Tile program (shared by all cores)."""
    wcols, offs = layout["wcols"], layout["offs"]
    chunks, groups = _device_plan(layout)
    total_cols = layout["total_cols"]
    out_cols = int(wcols.sum())

    nc = bacc.Bacc(None)
    comb_d = nc.declare_dram_parameter("comb", [128, total_cols], F16,
                                       isOutput=False)
    out_d = nc.declare_dram_parameter("out", [128, out_cols], F16,
                                      isOutput=True)

    with ExitStack() as ctx:
        tc = ctx.enter_context(tile.TileContext(nc))
        c_pool = ctx.enter_context(tc.tile_pool(name="c", bufs=len(chunks)))
        o_pool = ctx.enter_context(tc.tile_pool(name="o", bufs=4))
        psum_pool = ctx.enter_context(tc.tile_pool(name="psum", bufs=6,
                                                   space="PSUM"))

        cts = []
        for ci, (p0, np_, c0, c1) in enumerate(chunks):
            ct = c_pool.tile([128, c1 - c0], F16, tag=f"ct{ci}",
                             name=f"ct{ci}")
            eng = nc.sync if ci % 2 == 0 else nc.scalar
            eng.dma_start(ct[:], comb_d[:, c0:c1])
            cts.append(ct)

        chunk_of = {}
        for ci, (p0, np_, c0, c1) in enumerate(chunks):
            for p in range(p0, p0 + np_):
                chunk_of[p] = (ci, c0)

        # store schedule: STORE_GROUPS[k] evac groups per store DMA (padded
        # with single-group stores if needed); big early stores amortize
        # descriptor overhead, tiny late ones shorten the post-evac tail.
        # Engines rotate gpsimd/sync/scalar; the final store always rides
        # the by-then-idle sync ring.
        sg = list(STORE_GROUPS)
        while sum(sg) < len(groups):
            sg.append(1)
        while sum(sg) > len(groups) and sg:
            if sg[-1] > 1:
                sg[-1] -= 1
            else:
                sg.pop()
        store_of = []
        for si, n in enumerate(sg):
            store_of += [si] * n
        store_engs = [nc.gpsimd, nc.sync, nc.scalar]

        ocol = 0
        ot = None
        ot_cols = 0
        ot_base = 0
        for gi, (g0, gnp, gcols) in enumerate(groups):
            acc = psum_pool.tile([128, PSUM_COLS], F32, tag="acc", name="acc")
            ccol = 0
            for p in range(g0, g0 + gnp):
                ci, c0 = chunk_of[p]
                ct = cts[ci]
                o = int(offs[p]) - c0
                wc = int(wcols[p])
                # ch A: PE quadrant (0,0), PSUM partitions 0:64
                nc.tensor.matmul(acc[0:64, ccol:ccol + wc],
                                 ct[0:64, o:o + BW],
                                 ct[0:64, o + BW:o + BW + wc],
                                 start=True, stop=True)
                # ch B: PE quadrant (64,64), PSUM partitions 64:128
                nc.tensor.matmul(acc[64:128, ccol:ccol + wc],
                                 ct[64:128, o:o + BW],
                                 ct[64:128, o + BW:o + BW + wc],
                                 start=True, stop=True)
                ccol += wc
            if ot is None:
                nsg = sg[store_of[gi]]
                ot = o_pool.tile([128, nsg * PSUM_COLS], F16,
                                 tag=f"ot{nsg}", name="ot")
                ot_cols = 0
                ot_base = ocol
            # evac: vector for the final group (lower PSUM access latency
            # on the critical tail), else alternate scalar/vector
            if gi == len(groups) - 1 or gi % 2 == 1:
                nc.vector.tensor_copy(ot[:, ot_cols:ot_cols + gcols],
                                      acc[:, :gcols])
            else:
                nc.scalar.copy(ot[:, ot_cols:ot_cols + gcols],
                               acc[:, :gcols])
            ot_cols += gcols
            ocol += gcols
            if gi == len(groups) - 1 or store_of[gi + 1] != store_of[gi]:
                si = store_of[gi]
                eng = (nc.sync if gi == len(groups) - 1
                       else store_engs[si % len(store_engs)])
                eng.dma_start(out_d[:, ot_base:ot_base + ot_cols],
                              ot[:, :ot_cols])
                ot = None

    nc.finalize()
    return nc


def run(inputs, trace=False, **run_kwargs):
    """Run on 8 NeuronCores. Returns (full_output, BassKernelResults)."""
    log_dt = np.asarray(inputs["log_dt"], np.float32)
    log_a_real = np.asarray(inputs["log_a_real"], np.float32)
    a_imag = np.asarray(inputs["a_imag"], np.float32)
    coeffs = np.asarray(inputs["coeffs"], np.float32)
    seq_len = int(inputs.get("sequence_length", L))
    assert log_dt.shape == (H,) and log_a_real.shape == (H, NPOLE)
    assert a_imag.shape == (H, NPOLE) and coeffs.shape == (NDIR, H, NPOLE, 2)
    assert seq_len == L, f"kernel is compiled for sequence_length={L}"

    combs, layout, chans_per_core = _host_prep(
        log_dt, log_a_real, a_imag, coeffs)
    nc = _build_module(layout)
    in_maps = [{"comb": combs[c]} for c in range(NCORES)]
    results = run_bass_kernel_spmd(nc, in_maps, list(range(NCORES)),
                                   trace=trace, **run_kwargs)

    wcols = layout["wcols"]
    out = np.zeros((NDIR, H, L), np.float32)
    for core in range(NCORES):
        o = np.asarray(results.results[core]["out"], np.float32)
        chans = chans_per_core[core]
        ocol = 0
        for p in range(PAIRS):
            wc = int(wcols[p])
            q0 = wc // 2
            for k in range(2):
                h = chans[2 * p + k]
                blk = o[64 * k:64 * k + 64, ocol:ocol + wc]
                blk = blk.reshape(BW, q0, 2)
                out[:, h, :q0 * BW] = blk.transpose(2, 1, 0).reshape(2, -1)
            ocol += wc
    return out, results


def kernel(**inputs):
    return run(inputs)[0]
